# revision 21
# baseline (speedup 1.0000x reference)
"""Trainium2 Bass kernel for nn_Attention_11579231830437.

Masked multi-head attention (ReLU'd QKV projections, additive key mask,
multiplicative query mask) followed by training-mode BatchNorm over (B, T).

Strategy: data-parallel over batch B across 8 NeuronCores (4 batches each).
The host compacts each batch's sequence to its valid (mask==1) positions —
both attention masks zero out the same positions, so attention is computed
only on the ~50% valid positions (sorted batch->slot assignment keeps the
per-slot padded length tight). BatchNorm mean/var sums are all-reduced
across cores on-chip; normalization is applied on-device before gathering.

On-device layout per batch slot (Tj = padded valid length of slot j):
  QT, KT   [C, Tj]   channel-major (per-partition bias + relu on ACT)
  V        [t, (h, dv | ones)]  row-major with a ones column per head, so the
           attention@V matmul also emits the softmax denominator row D.
  scores   ST[s, t] = KhT.T @ QhT per head (fp32r, row-group tiled, K=32)
  P        exp(scale*ST + key_bias[s])  (key mask folded into the exp bias)
  out      O'[dv, t] accumulated over s-chunks; x = O' * (qmask/D) broadcast
           via a block-ones matmul; BN stats accumulate via fused reduces.
"""

import os
import sys
import types
from contextlib import ExitStack

import numpy as np

# Defensive: concourse.bass_utils imports antenv.axon_hooks when tracing is
# requested via env; provide a no-op holder if the image lacks it.
try:
    import antenv.axon_hooks  # noqa: F401
except Exception:
    try:
        import antenv

        _m = types.ModuleType("antenv.axon_hooks")
        _m._hook = None
        _m.set_axon_ntff_profile_hook = lambda h: setattr(_m, "_hook", h)
        _m.get_axon_ntff_profile_hook = lambda: getattr(_m, "_hook", None)
        sys.modules["antenv.axon_hooks"] = _m
        antenv.axon_hooks = _m
    except Exception:
        pass

import concourse.bass as bass
import concourse.bacc as bacc
import concourse.tile as tile
from concourse import mybir
from concourse.bass_utils import run_bass_kernel_spmd

f32 = mybir.dt.float32
f32r = mybir.dt.float32r
bf16 = mybir.dt.bfloat16
FT = mybir.ActivationFunctionType
ALU = mybir.AluOpType

N_CORES = 8
B, C, T, H = 32, 256, 512, 8
D = C // H                      # 32 per-head dim
NB = B // N_CORES               # 4 batch slots per core
G = C // 128                    # 2 channel chunks of 128
EPS = 1e-5
SCALE = 1.0 / float(np.sqrt(D))
KB_NEG = -200.0                 # exp(-200 + few) == 0.0 in fp32
INV_BT = 1.0 / float(B * T)


def _cdiv(a, b):
    return (a + b - 1) // b


def _build(slot_T):
    """Build the SPMD program for per-slot padded lengths slot_T (NB ints)."""
    STAGE = int(os.environ.get("K_STAGE", "9"))  # 1 proj, 2 scores, 3 av, 9 full
    USE_CC = os.environ.get("K_NO_CC", "") == ""
    Tmax = max(slot_T)
    nc = bacc.Bacc("TRN2", target_bir_lowering=False, debug=False,
                   num_devices=N_CORES)

    seq_d, kb_d, qm_d, out_d = [], [], [], []
    for j in range(NB):
        Tj = slot_T[j]
        sch = _cdiv(Tj, 128)
        seq_d.append(nc.dram_tensor(f"seq{j}", [C, Tj], f32,
                                    kind="ExternalInput").ap())
        kb_d.append(nc.dram_tensor(f"kb{j}", [sch * 128], f32,
                                   kind="ExternalInput").ap())
        qm_d.append(nc.dram_tensor(f"qm{j}", [1, Tj], f32,
                                   kind="ExternalInput").ap())
        out_d.append(nc.dram_tensor(f"out{j}", [C, Tj], f32,
                                    kind="ExternalOutput").ap())

    wq_d = nc.dram_tensor("wqt", [C, C], f32, kind="ExternalInput").ap()
    wk_d = nc.dram_tensor("wkt", [C, C], f32, kind="ExternalInput").ap()
    wv_d = nc.dram_tensor("wvt", [C, C], f32, kind="ExternalInput").ap()
    bq_d = nc.dram_tensor("bq", [C, 1], f32, kind="ExternalInput").ap()
    bk_d = nc.dram_tensor("bk", [C, 1], f32, kind="ExternalInput").ap()
    bvr_d = nc.dram_tensor("bvr", [1, C], f32, kind="ExternalInput").ap()
    gm_d = nc.dram_tensor("gamma", [C, 1], f32, kind="ExternalInput").ap()
    bt_d = nc.dram_tensor("beta", [C, 1], f32, kind="ExternalInput").ap()
    bo_d = nc.dram_tensor("bones", [128, 128], f32, kind="ExternalInput").ap()
    on_d = nc.dram_tensor("ones2", [1, 2 * H], f32, kind="ExternalInput").ap()

    with tile.TileContext(nc) as tc, ExitStack() as ctx:
        const = ctx.enter_context(tc.tile_pool(name="const", bufs=1))
        seqp = ctx.enter_context(tc.tile_pool(name="seqp", bufs=2))
        qkp = ctx.enter_context(tc.tile_pool(name="qkp", bufs=2))
        vp = ctx.enter_context(tc.tile_pool(name="vp", bufs=2))
        pp = ctx.enter_context(tc.tile_pool(name="pp", bufs=2))
        xp = ctx.enter_context(tc.tile_pool(name="xp", bufs=NB))
        smallp = ctx.enter_context(tc.tile_pool(name="smallp", bufs=2))
        fbp = ctx.enter_context(tc.tile_pool(name="fbp", bufs=2))
        statp = ctx.enter_context(tc.tile_pool(name="statp", bufs=1))
        outp = ctx.enter_context(tc.tile_pool(name="outp", bufs=4))
        scrp = ctx.enter_context(tc.tile_pool(name="scrp", bufs=2))
        ps_proj = ctx.enter_context(tc.tile_pool(name="ps_proj", bufs=1, space="PSUM"))
        ps_sc = ctx.enter_context(tc.tile_pool(name="ps_sc", bufs=1, space="PSUM"))
        ps_av = ctx.enter_context(tc.tile_pool(name="ps_av", bufs=2, space="PSUM"))
        ps_fb = ctx.enter_context(tc.tile_pool(name="ps_fb", bufs=1, space="PSUM"))
        dramp = ctx.enter_context(tc.tile_pool(name="dramp", bufs=1, space="DRAM"))

        # ---- constants ----
        wq_s = [const.tile([128, C], f32r, tag=f"wq{k}", name=f"wq{k}") for k in range(G)]
        wk_s = [const.tile([128, C], f32r, tag=f"wk{k}", name=f"wk{k}") for k in range(G)]
        wv_s = [const.tile([128, C], f32r, tag=f"wv{k}", name=f"wv{k}") for k in range(G)]
        for k in range(G):
            nc.sync.dma_start(wq_s[k][:], wq_d[128 * k:128 * (k + 1), :].bitcast(f32r))
            nc.sync.dma_start(wk_s[k][:], wk_d[128 * k:128 * (k + 1), :].bitcast(f32r))
            nc.sync.dma_start(wv_s[k][:], wv_d[128 * k:128 * (k + 1), :].bitcast(f32r))
        bq_t = const.tile([128, G], f32, tag="bq")
        bk_t = const.tile([128, G], f32, tag="bk")
        gm_t = const.tile([128, G], f32, tag="gm")
        bt_t = const.tile([128, G], f32, tag="bt")
        for td, ts in ((bq_d, bq_t), (bk_d, bk_t), (gm_d, gm_t), (bt_d, bt_t)):
            src = bass.AP(tensor=td.tensor, offset=td.offset,
                          ap=[[1, 128], [128, G]])
            nc.sync.dma_start(ts[:], src)
        bv_b = const.tile([128, C], f32, tag="bvb")
        nc.sync.dma_start(
            bv_b[:],
            bass.AP(tensor=bvr_d.tensor, offset=bvr_d.offset,
                    ap=[[0, 128], [1, C]]),
        )
        bones_b = const.tile([128, 128], f32r, tag="bones")
        nc.sync.dma_start(bones_b[:], bo_d[:].bitcast(f32r))
        eps_t = const.tile([128, 1], f32, tag="eps")
        nc.vector.memset(eps_t[:], EPS)
        ones2_b = const.tile([128, H, 2], f32, tag="ones2b")
        nc.sync.dma_start(
            ones2_b[:],
            bass.AP(tensor=on_d.tensor, offset=on_d.offset,
                    ap=[[0, 128], [2, H], [1, 2]]),
        )

        st1 = [statp.tile([128, NB], f32, tag=f"st1_{g}", name=f"st1_{g}") for g in range(G)]
        st2 = [statp.tile([128, NB], f32, tag=f"st2_{g}", name=f"st2_{g}") for g in range(G)]

        xts = []  # [sl][g] -> XT tile
        for sl in range(NB):
            Tj = slot_T[sl]
            sch = _cdiv(Tj, 128)
            msz = [min(128, Tj - 128 * i) for i in range(sch)]

            # ---- phase 1: projections ----
            s_in = [seqp.tile([128, Tmax], f32r, tag=f"sin{k}", name=f"sin{k}_{sl}") for k in range(G)]
            for k in range(G):
                nc.sync.dma_start(s_in[k][:, :Tj],
                                  seq_d[sl][128 * k:128 * (k + 1), :].bitcast(f32r))
            kb_t = smallp.tile([128, 4], f32, tag="kb")
            nc.scalar.dma_start(
                kb_t[:, :sch],
                bass.AP(tensor=kb_d[sl].tensor, offset=kb_d[sl].offset,
                        ap=[[1, 128], [128, sch]]),
            )
            qm_b = smallp.tile([128, Tmax], f32, tag="qmb", name=f"qmb_{sl}")
            nc.scalar.dma_start(
                qm_b[:, :Tj],
                bass.AP(tensor=qm_d[sl].tensor, offset=qm_d[sl].offset,
                        ap=[[0, 128], [1, Tj]]),
            )

            qt = [qkp.tile([128, Tmax], f32r, tag=f"qt{g}", name=f"qt{g}_{sl}") for g in range(G)]
            kt = [qkp.tile([128, Tmax], f32r, tag=f"kt{g}", name=f"kt{g}_{sl}") for g in range(G)]
            for g in range(G):
                for (w_s, b_t, dst) in ((wq_s, bq_t, qt), (wk_s, bk_t, kt)):
                    psq = ps_proj.tile([128, 512], f32, tag="psproj")
                    for k in range(G):
                        nc.tensor.matmul(
                            psq[:, :Tj],
                            w_s[k][:, 128 * g:128 * (g + 1)],
                            s_in[k][:, :Tj],
                            start=(k == 0), stop=(k == G - 1),
                        )
                    nc.scalar.activation(
                        dst[g][:, :Tj], psq[:, :Tj], FT.Relu,
                        bias=b_t[:, g:g + 1], scale=1.0,
                    )

            v_t = [vp.tile([128, H, D + 2], bf16, tag=f"vt{i}", name=f"vt{i}_{sl}") for i in range(sch)]
            for i in range(sch):
                m = msz[i]
                psv = ps_proj.tile([128, 512], f32, tag="psproj")
                for k in range(G):
                    nc.tensor.matmul(
                        psv[:m, :C],
                        s_in[k][:, 128 * i:128 * i + m],
                        wv_s[k][:],
                        start=(k == 0), stop=(k == G - 1),
                    )
                vw = v_t[i][:m, :, 0:D]
                nc.vector.tensor_tensor(
                    out=vw,
                    in0=psv[:m, :C].rearrange("p (h d) -> p h d", h=H),
                    in1=bv_b[:m, :].rearrange("p (h d) -> p h d", h=H),
                    op=ALU.add,
                )
                nc.vector.tensor_scalar_max(out=vw, in0=vw, scalar1=0.0)
                nc.vector.tensor_copy(v_t[i][:m, :, D:D + 2], ones2_b[:m, :, :])

            # ---- phase 2: scores + exp ----
            p_t = [[None] * G for _ in range(sch)]
            if STAGE < 2:
                for i in range(sch):
                    for g in range(G):
                        pt = pp.tile([128, 4, Tmax], bf16, tag=f"p{i}{g}", name=f"pz{i}{g}_{sl}")
                        nc.vector.memset(pt[:], 0.5)
                        p_t[i][g] = pt
            for i in range(sch if STAGE >= 2 else 0):
                m = msz[i]
                for g in range(G):
                    ps4 = ps_sc.tile([128, 4, 512], f32, tag="pssc")
                    for j in range(4):
                        nc.tensor.matmul(
                            ps4[:m, j, :Tj],
                            kt[g][32 * j:32 * (j + 1), 128 * i:128 * i + m],
                            qt[g][32 * j:32 * (j + 1), :Tj],
                            start=True, stop=True,
                            tile_position=(32 * j, 0),
                        )
                    pt = pp.tile([128, 4, Tmax], bf16, tag=f"p{i}{g}", name=f"p{i}{g}_{sl}")
                    nc.scalar.activation(
                        pt[:m, :, :Tj], ps4[:m, :, :Tj], FT.Exp,
                        bias=kb_t[:m, i:i + 1], scale=SCALE,
                    )
                    p_t[i][g] = pt

            # ---- phase 3: att @ V, normalize, assemble ----
            xt = [xp.tile([128, Tmax], f32, tag=f"xt{g}", name=f"xt{g}_{sl}") for g in range(G)]
            xts.append(xt)
            # D rows of all 8 head-pairs parked at partitions (32r, 32r+1);
            # one batched reciprocal serves every pair in this slot.
            d_all = smallp.tile([128, Tmax], f32, tag="dall", name=f"dall_{sl}")
            r_all = smallp.tile([128, Tmax], f32, tag="rall", name=f"rall_{sl}")
            f_all = smallp.tile([128, Tmax], f32r, tag="fall", name=f"fall_{sl}")
            a_sbs = {}
            for g in range(G):
                for p in range(2):
                    r = 2 * g + p
                    psA = ps_av.tile([128, 512], f32, tag="psav", name=f"psav{g}{p}_{sl}")
                    for pp_ in range(2):
                        h = 4 * g + 2 * p + pp_
                        base = 64 * pp_
                        for i in range(sch):
                            m = msz[i]
                            nc.tensor.matmul(
                                psA[base:base + D + 2, :Tj],
                                v_t[i][:m, h, :],
                                p_t[i][g][:m, 2 * p + pp_, :Tj],
                                start=(i == 0), stop=(i == sch - 1),
                                tile_position=(0, base),
                            )
                    asb = fbp.tile([128, Tmax], f32, tag=f"asb{r}",
                                   name=f"asb{r}_{sl}")
                    a_sbs[r] = asb
                    nc.vector.tensor_copy(asb[:, :Tj], psA[:, :Tj])
                    nc.gpsimd.dma_start(d_all[32 * r:32 * r + 1, :Tj],
                                        asb[D:D + 1, :Tj])
                    nc.gpsimd.dma_start(d_all[32 * r + 1:32 * r + 2, :Tj],
                                        asb[64 + D:64 + D + 1, :Tj])
            nc.vector.tensor_scalar_add(out=d_all[:, :Tj], in0=d_all[:, :Tj],
                                        scalar1=1e-6)
            nc.vector.reciprocal(r_all[:, :Tj], d_all[:, :Tj])
            nc.vector.tensor_tensor(out=f_all[:, :Tj], in0=r_all[:, :Tj],
                                    in1=qm_b[:, :Tj], op=ALU.mult)
            for g in range(G):
                for p in range(2):
                    r = 2 * g + p
                    asb = a_sbs[r]
                    psF = ps_fb.tile([128, 512], f32, tag="psfb",
                                     name=f"psF_{r}_{sl}")
                    nc.tensor.matmul(psF[:, :Tj],
                                     bones_b[32 * r:32 * r + 2, :],
                                     f_all[32 * r:32 * r + 2, :Tj],
                                     start=True, stop=True,
                                     tile_position=(32 * r, 0))
                    for pp_ in range(2):
                        j = 2 * p + pp_
                        nc.vector.scalar_tensor_tensor(
                            out=xt[g][32 * j:32 * (j + 1), :Tj],
                            in0=asb[64 * pp_:64 * pp_ + D, :Tj],
                            scalar=1.0,
                            in1=psF[64 * pp_:64 * pp_ + D, :Tj],
                            op0=ALU.mult, op1=ALU.mult,
                            accum_out=st1[g][32 * j:32 * (j + 1), sl:sl + 1],
                        )
                scr = scrp.tile([128, Tmax], f32, tag="scr")
                nc.vector.tensor_tensor(
                    out=scr[:, :Tj], in0=xt[g][:, :Tj], in1=xt[g][:, :Tj],
                    op=ALU.mult,
                )
                nc.vector.tensor_reduce(
                    st2[g][:, sl:sl + 1], scr[:, :Tj],
                    axis=mybir.AxisListType.X, op=ALU.add,
                )

        # ---- phase 4: BN all-reduce + apply ----
        cc_sb = statp.tile([128, 2 * G], f32, tag="ccsb")
        for g in range(G):
            nc.vector.tensor_reduce(cc_sb[:, g:g + 1], st1[g][:],
                                    axis=mybir.AxisListType.X, op=ALU.add)
            nc.vector.tensor_reduce(cc_sb[:, G + g:G + g + 1], st2[g][:],
                                    axis=mybir.AxisListType.X, op=ALU.add)
        cc_in = dramp.tile([128, 2 * G], f32, tag="ccin")
        cc_out = dramp.tile([128, 2 * G], f32, tag="ccout")
        nc.sync.dma_start(cc_in[:], cc_sb[:])
        if USE_CC:
            nc.gpsimd.collective_compute(
                "AllReduce", ALU.add,
                replica_groups=[list(range(N_CORES))],
                ins=[cc_in[:]], outs=[cc_out[:]],
            )
        else:
            nc.sync.dma_start(cc_out[:], cc_in[:])
        red = statp.tile([128, 2 * G], f32, tag="red")
        nc.sync.dma_start(red[:], cc_out[:])

        a_g, bs_g = [], []
        for g in range(G):
            mean = statp.tile([128, 1], f32, tag=f"mean{g}")
            nc.vector.tensor_scalar_mul(out=mean[:], in0=red[:, g:g + 1],
                                        scalar1=INV_BT)
            var = statp.tile([128, 1], f32, tag=f"var{g}")
            # var = E[x^2] - mean^2 = red2*INV_BT - mean*mean
            nc.vector.scalar_tensor_tensor(
                out=var[:], in0=mean[:], scalar=-1.0, in1=mean[:],
                op0=ALU.mult, op1=ALU.mult,
            )
            nc.vector.scalar_tensor_tensor(
                out=var[:], in0=red[:, G + g:G + g + 1], scalar=INV_BT,
                in1=var[:], op0=ALU.mult, op1=ALU.add,
            )
            sd = statp.tile([128, 1], f32, tag=f"sd{g}")
            nc.scalar.activation(sd[:], var[:], FT.Sqrt, bias=eps_t[:],
                                 scale=1.0)
            rs = statp.tile([128, 1], f32, tag=f"rs{g}")
            nc.vector.reciprocal(rs[:], sd[:])
            a = statp.tile([128, 1], f32, tag=f"a{g}")
            nc.vector.tensor_tensor(out=a[:], in0=gm_t[:, g:g + 1], in1=rs[:],
                                    op=ALU.mult)
            bs = statp.tile([128, 1], f32, tag=f"bs{g}")
            nc.vector.scalar_tensor_tensor(
                out=bs[:], in0=mean[:], scalar=-1.0, in1=a[:],
                op0=ALU.mult, op1=ALU.mult,
            )
            nc.vector.tensor_tensor(out=bs[:], in0=bt_t[:, g:g + 1], in1=bs[:],
                                    op=ALU.add)
            a_g.append(a)
            bs_g.append(bs)

        for sl in range(NB):
            Tj = slot_T[sl]
            for g in range(G):
                ot = outp.tile([128, Tmax], f32, tag="ot")
                nc.vector.tensor_scalar(
                    out=ot[:, :Tj], in0=xts[sl][g][:, :Tj],
                    scalar1=a_g[g][:], scalar2=bs_g[g][:],
                    op0=ALU.mult, op1=ALU.add, accum_out=None,
                )
                nc.gpsimd.dma_start(out_d[sl][128 * g:128 * (g + 1), :Tj],
                                    ot[:, :Tj])

    nc.compile()
    return nc


_CACHE = {}


def _get_program(slot_T):
    key = tuple(slot_T)
    if key not in _CACHE:
        _CACHE[key] = _build(list(key))
    return _CACHE[key]


def kernel(seq, mask, Wq, bq, Wk, bk, Wv, bv, gamma, beta):
    seq = np.ascontiguousarray(np.asarray(seq, dtype=np.float32))
    mask_np = np.asarray(mask)
    counts = (mask_np != 0).sum(axis=1).astype(np.int64)
    order = np.argsort(-counts, kind="stable")

    # slot j on core c handles batch order[8*j + c]
    slot_T = []
    for j in range(NB):
        mx = int(counts[order[N_CORES * j:N_CORES * (j + 1)]].max())
        mx = (mx + 1) // 2 * 2  # fp32r matmuls need even free sizes
        slot_T.append(min(T, max(256, mx)))

    nc = _get_program(slot_T)

    wqt = np.ascontiguousarray(np.asarray(Wq, np.float32).T)
    wkt = np.ascontiguousarray(np.asarray(Wk, np.float32).T)
    wvt = np.ascontiguousarray(np.asarray(Wv, np.float32).T)
    bq_c = np.ascontiguousarray(np.asarray(bq, np.float32).reshape(C, 1))
    bk_c = np.ascontiguousarray(np.asarray(bk, np.float32).reshape(C, 1))
    bvr = np.ascontiguousarray(np.asarray(bv, np.float32).reshape(1, C))
    gm = np.ascontiguousarray(np.asarray(gamma, np.float32).reshape(C, 1))
    bt = np.ascontiguousarray(np.asarray(beta, np.float32).reshape(C, 1))
    bones = np.zeros((128, 128), np.float32)
    for r in range(4):
        bones[32 * r, 0:32] = 1.0
        bones[32 * r + 1, 64:96] = 1.0
    ones2 = np.tile(np.array([[1.0, 0.0]], np.float32), (1, H))

    idx_map = {}
    in_maps = []
    for c in range(N_CORES):
        m = {
            "wqt": wqt, "wkt": wkt, "wvt": wvt,
            "bq": bq_c, "bk": bk_c, "bvr": bvr,
            "gamma": gm, "beta": bt, "bones": bones, "ones2": ones2,
        }
        for j in range(NB):
            Tj = slot_T[j]
            sch = _cdiv(Tj, 128)
            b = int(order[N_CORES * j + c])
            idx = np.flatnonzero(mask_np[b] != 0)
            n = len(idx)
            idx_map[(c, j)] = (b, idx)
            sc = np.zeros((C, Tj), np.float32)
            sc[:, :n] = seq[b][:, idx]
            kb = np.full(sch * 128, KB_NEG, np.float32)
            kb[:n] = 0.0
            qm = np.zeros((1, Tj), np.float32)
            qm[:, :n] = 1.0
            m[f"seq{j}"] = sc
            m[f"kb{j}"] = kb
            m[f"qm{j}"] = qm
        in_maps.append(m)

    global _last_in_maps
    _last_in_maps = in_maps
    res = run_bass_kernel_spmd(nc, in_maps, core_ids=list(range(N_CORES)))

    out = np.zeros((B, C, T), np.float32)
    for c in range(N_CORES):
        for j in range(NB):
            b, idx = idx_map[(c, j)]
            n = len(idx)
            if n:
                out[b][:, idx] = res.results[c][f"out{j}"][:, :n]
    return out
